# revision 25
# baseline (speedup 1.0000x reference)
"""TRN2 Bass kernel for nn_MultiHeadAttention_50835232916148 (fp8 rewrite).

Pre-LN MHA block (HS=1024, 16 heads, bs=8, sl=1024), data-parallel over
batch across 8 NeuronCores (bs=1 per core, no collectives).

v2 design (vs the bf16 baseline at ~284us):
- All big matmuls (QKV proj, V proj, ctx, out-proj, LN stats) run as fp8e4
  DoubleRow pairs: one instruction contracts 2 k-tiles (256 deep) in the
  time bf16 streams one, measured 2.37x per-instruction on HW.  Weights are
  host-scaled x32 (x16 for out-proj) into e4m3's normal range; the inverse
  scale rides the psum->SBUF bias ops as a free immediate multiplier.
- Scores stay bf16 (64-deep contraction is column-bound; fp8 DR measured
  slower in that shape).
- The exp stream is split: 5 of 8 key-tiles per (hp, chunk) run on the ACT
  engine (exact exp, fp8 out), 3 run on DVE as a one-op "Schraudolph-fp8":
  bits = RNE(score*8/ln2 + 55.54 + mask_bias) computed by tensor_scalar
  into uint8 = the e4m3 bit pattern of ~exp(score) (3% rms).  Negative
  saturation gives masked keys exactly 0.
- Softmax denominator rides the ctx matmul as a 65th vaug column of 1/256
  (so the reciprocal lands in e4m3/bf16 sweet spot); recip is one DVE
  reciprocal per head-pair on DRAM-gathered rows; normalize muls run on
  the otherwise-idle Pool engine (SBUF-only: GPSIMD cannot touch PSUM).
- out_proj bias + residual are host-folded into the fp32 xt tensor, so the
  out-proj tail is one scalar_tensor_tensor per tile.

Per-core dataflow ([feature, token] transposed activations):
  x8 (fp8, from host) --fp8-DR ones-matmul stats--> istd, b2
  y = x8*bcast(istd) + bcast(b2)                      [d,t] fp8
  vaug[t, h*65:(h+1)*65] = [y.T@Wv + bv | 1/256]      token-major fp8
  per head-pair hp: qb,kb = WqkT.T @ y + b (DR, interleaved into hp-1)
    per (n, jt): scoresT = [kbA^T qbA | kbB^T qbB]    wide 2-bank PSUM
                 pt[jt&1] = exp-ish(scoresT + mask)   ACT or DVE-schraudolph
                 per pair p: ctx_aug += vaug[2p:2p+2].T @ pt-pair (fp8 DR)
    recip = DVE reciprocal on DRAM-gathered denom rows; rb = bcast DMA
    ctxn = cs * rb on Pool (fp8 out, head B shifted to partitions 64-127)
  outT = WoutT.T @ ctxn (DR); out = outT*2^-12 + (xT + bo)  one DVE stt
"""

import numpy as np

import concourse.bass as bass
import concourse.mybir as mybir
import concourse.tile as tile
from concourse.bass_utils import run_bass_kernel_spmd

P = 128
HS = 1024
SL = 1024
NHEAD = 16
DH = 64
BS = 8
NT = HS // P          # 8 feature/token tiles
TC = 512              # matmul free-dim chunk (fp32 PSUM bank)
NCH = SL // TC        # 2
LN_EPS = 1e-5
MASK_NEG = -1e8
F32 = mybir.dt.float32
F32R = mybir.dt.float32r
BF16 = mybir.dt.bfloat16
FP8 = mybir.dt.float8e4
U8 = mybir.dt.uint8
AF = mybir.ActivationFunctionType
ALU = mybir.AluOpType
DRM = mybir.MatmulPerfMode.DoubleRow

WS_QKV = 32.0         # host scale on wqkv (e4m3 normal range)
WS_OUT = 16.0         # host scale on wout
ONES_COL = 1.0 / 256.0  # vaug denominator column value
OUT_SCALE = 1.0 / (WS_OUT * 256.0)
LOG2E8 = 8.0 / np.log(2.0)   # schraudolph slope
SC_C = 56.0 - 0.46           # schraudolph intercept (calibrated)
SJT = (3, 7)                 # jt slots whose exp runs on DVE


def _hoist_waits(nc):
    """walrus in this env rejects >1 inline wait per instruction and ANY
    inline wait on Matmult; hoist them onto single-wait NoOps."""
    n_fixed = 0
    for _, bb in nc.bb_map.items():
        inner = bb.bb
        insts = inner.instructions
        new = []
        changed = False
        for inst in insts:
            si = getattr(inst, "sync_info", None)
            if si is not None and si.on_wait:
                keep = 0 if isinstance(inst, mybir.InstMatmult) else 1
                waits = list(si.on_wait)
                if len(waits) > keep:
                    kept = waits[-keep:] if keep else []
                    for w in waits[: len(waits) - keep]:
                        new.append(
                            mybir.InstNoOp(
                                name=nc.get_next_instruction_name(),
                                sync_info=mybir.SyncInfo(on_wait=[w], on_update=[]),
                                bass_nofuse=True,
                                engine=inst.engine,
                            )
                        )
                    inst.sync_info = mybir.SyncInfo(
                        on_wait=kept, on_update=list(si.on_update)
                    )
                    n_fixed += 1
                    changed = True
            new.append(inst)
        if changed:
            inner.instructions = new
    return n_fixed


def _build_nc(hoist=True):
    nc = bass.Bass()

    xt = nc.dram_tensor("xt", [HS, SL], BF16, kind="ExternalInput")
    x8d = nc.dram_tensor("x8d", [HS, SL], FP8, kind="ExternalInput")
    wqkv = nc.dram_tensor("wqkv", [HS, 3 * HS], FP8, kind="ExternalInput")
    wout = nc.dram_tensor("wout", [HS, HS], FP8, kind="ExternalInput")
    bqk = nc.dram_tensor("bqk", [P, 16], F32, kind="ExternalInput")
    mb = nc.dram_tensor("mb", [P, NT], F32, kind="ExternalInput")
    mbs = nc.dram_tensor("mbs", [P, NT], F32, kind="ExternalInput")
    onesr = nc.dram_tensor("onesr", [1, P], BF16, kind="ExternalInput")
    epsr = nc.dram_tensor("epsr", [1, 1], F32, kind="ExternalInput")
    out = nc.dram_tensor("out", [HS, SL], BF16, kind="ExternalOutput")
    sden = [nc.dram_tensor(f"sden{h}", [4, TC], BF16, kind="Internal")
            for h in range(NHEAD // 2)]
    srec = [nc.dram_tensor(f"srec{h}", [4, TC], BF16, kind="Internal")
            for h in range(NHEAD // 2)]

    with tile.TileContext(nc) as tc, nc.allow_low_precision(
            reason="fp8 matmuls; tolerance is 2e-2 and residual is fp32"):
        with (
            tc.tile_pool(name="big", bufs=1) as big,
            tc.tile_pool(name="wstream", bufs=6) as wstream,
            tc.tile_pool(name="scratch", bufs=2) as scratch,
            tc.tile_pool(name="qks", bufs=4) as qks,
            tc.tile_pool(name="pts", bufs=4) as pts,
            tc.tile_pool(name="stream", bufs=3) as stream,
            tc.tile_pool(name="vecs", bufs=1) as vecs,
            tc.tile_pool(name="csp", bufs=2) as csp,
            tc.tile_pool(name="rbp", bufs=4) as rbp,
            tc.tile_pool(name="denp", bufs=2) as denp,
            tc.tile_pool(name="consts", bufs=1) as consts,
            tc.tile_pool(name="wide", bufs=2, space="PSUM") as wide,
            tc.tile_pool(name="acc", bufs=4, space="PSUM") as acc,
        ):
            # ---- big activation tiles ----
            t_xb = big.tile([P, NT, SL], FP8, tag="xb")
            t_x16 = big.tile([P, NT, SL], BF16, tag="x16")
            t_sq = big.tile([P, NT, SL], BF16, tag="sq")
            t_y = big.tile([P, NT, SL], FP8, tag="y")
            VST = DH + 2  # head stride in vaug (even, for fp8 dual-load)
            t_vaug = big.tile([P, NT, NHEAD * VST], FP8, tag="vaug")
            t_ctxn = big.tile([P, NT, SL], FP8, tag="ctxn")
            wv_big = big.tile([P, NT, HS], FP8, tag="wv")

            # PE clock warmup (HAM): memset-fed, no DMA deps
            c_oneb = consts.tile([P, 1], BF16, tag="oneb")
            nc.vector.memset(c_oneb[:], 1.0)
            ps_warm = acc.tile([1, TC], F32, tag="acc", name="warmps")
            for _ in range(60):
                nc.tensor.matmul(ps_warm[:, 0:1], c_oneb[:], c_oneb[:],
                                 start=True, stop=True)

            # ---- input DMA staging ----
            # first x8 tiles split across queues so stats start early
            for i in range(NT):
                nc.sync.dma_start(t_xb[:, i, 0:TC],
                                  x8d[i * P:(i + 1) * P, 0:TC])
                nc.gpsimd.dma_start(t_xb[:, i, TC:SL],
                                    x8d[i * P:(i + 1) * P, TC:SL])
            for k in range(NT):
                [nc.sync, nc.gpsimd][k % 2].dma_start(
                    wv_big[:, k, :], wqkv[k * P:(k + 1) * P, 2 * HS:3 * HS])
            c_bqk = consts.tile([P, 16], F32, tag="bqk")
            nc.sync.dma_start(c_bqk[:], bqk[:])
            c_mb = consts.tile([P, NT], F32, tag="mb")
            nc.sync.dma_start(c_mb[:], mb[:])
            c_mbs = consts.tile([P, NT], F32, tag="mbs")
            nc.gpsimd.dma_start(c_mbs[:], mbs[:])
            c_or = consts.tile([1, P], BF16, tag="onesr")
            nc.gpsimd.dma_start(c_or[:], onesr[:])
            c_or65 = consts.tile([DH + 1, P], BF16, tag="or65")
            nc.gpsimd.dma_start(c_or65[DH:DH + 1, :], onesr[:])
            c_eps = consts.tile([1, 1], F32, tag="eps")
            nc.sync.dma_start(c_eps[:], epsr[:])

            # ================= Phase 1: LayerNorm =================
            # ACT converts fp8 x -> bf16 (full speed there); sq/stats/y all
            # run in fast bf16 paths; y goes back to fp8 via ACT copies.
            for i in range(NT):
                nc.scalar.activation(t_x16[:, i, :], t_xb[:, i, :], AF.Copy)
            # ACT exp-table load parked behind the x conversions
            t_warm = vecs.tile([1, 1], F32, tag="warm")
            nc.vector.memset(t_warm[:], 1.0)
            nc.scalar.activation(t_warm[:], t_warm[:], AF.Exp)
            st_m = wide.tile([1, SL], F32, tag="wide", name="stm")
            st_s = wide.tile([1, SL], F32, tag="wide", name="sts")
            for i in range(NT):
                nc.vector.tensor_mul(t_sq[:, i, :], t_x16[:, i, :],
                                     t_x16[:, i, :])
            for i in range(NT):
                for n in range(NCH):
                    sl_ = slice(n * TC, (n + 1) * TC)
                    nc.tensor.matmul(st_m[:, sl_], c_oneb[:],
                                     t_x16[:, i, sl_],
                                     start=(i == 0), stop=(i == NT - 1))
                    nc.tensor.matmul(st_s[:, sl_], c_oneb[:],
                                     t_sq[:, i, sl_],
                                     start=(i == 0), stop=(i == NT - 1))
            # LN tail chunked per 512-column half (baseline-proven chain)
            v_mean = vecs.tile([1, SL], F32, tag="mean")
            v_msq = vecs.tile([1, SL], F32, tag="msq")
            v_tmp = vecs.tile([1, SL], F32, tag="tmp")
            v_lnv = vecs.tile([1, SL], F32, tag="lnv")
            v_istd = vecs.tile([1, SL], BF16, tag="istd")
            v_b2 = vecs.tile([1, SL], BF16, tag="b2")
            t_A = scratch.tile([P, SL], BF16, tag="ab", name="tA")
            t_B = scratch.tile([P, SL], BF16, tag="ab", name="tB")
            for c in range(NCH):
                cl = slice(c * TC, (c + 1) * TC)
                nc.scalar.activation(v_msq[:, cl], st_s[:, cl], AF.Copy,
                                     scale=1.0 / HS)
                nc.scalar.activation(v_mean[:, cl], st_m[:, cl], AF.Copy,
                                     scale=1.0 / HS)
                nc.vector.tensor_mul(v_tmp[:, cl], v_mean[:, cl],
                                     v_mean[:, cl])
                nc.vector.tensor_sub(v_msq[:, cl], v_msq[:, cl],
                                     v_tmp[:, cl])   # -> var
                nc.scalar.activation(v_lnv[:, cl], v_msq[:, cl], AF.Ln,
                                     bias=c_eps[:])
                nc.scalar.activation(v_istd[:, cl], v_lnv[:, cl], AF.Exp,
                                     scale=-0.5)
                nc.vector.scalar_tensor_tensor(v_b2[:, cl], v_mean[:, cl],
                                               -1.0, v_istd[:, cl],
                                               ALU.mult, ALU.mult)
                pA = acc.tile([P, TC], F32, tag="acc", name=f"pA{c}")
                nc.tensor.matmul(pA[:], c_or[:], v_istd[:, cl],
                                 start=True, stop=True)
                nc.vector.tensor_copy(t_A[:, cl], pA[:])
                pB = acc.tile([P, TC], F32, tag="acc", name=f"pB{c}")
                nc.tensor.matmul(pB[:], c_or[:], v_b2[:, cl],
                                 start=True, stop=True)
                nc.vector.tensor_copy(t_B[:, cl], pB[:])
            # y: bf16 affine on DVE, fp8 via ACT copy, i-major
            for i in range(NT):
                yb = stream.tile([P, SL], BF16, tag="yb", bufs=3,
                                 name=f"yb{i}")
                for c in range(NCH):
                    cl = slice(c * TC, (c + 1) * TC)
                    t1 = stream.tile([P, TC], BF16, tag="t1v", bufs=3,
                                     name=f"yt{i}_{c}")
                    nc.vector.tensor_mul(t1[:], t_x16[:, i, cl], t_A[:, cl])
                    nc.vector.tensor_add(yb[:, cl], t1[:], t_B[:, cl])
                nc.scalar.activation(t_y[:, i, :], yb[:], AF.Copy)

            # ========= Phase 3+4: per head-pair QK proj + attention =====
            def normalize(hp, allcs):
                """allcs = [65, 4, TC] bf16; col r = 2*dn+hh.  Gather denom
                rows through DRAM, one DVE reciprocal, bcast back, Pool
                muls into ctxn (fp8)."""
                nc.sync.dma_start(sden[hp][:], allcs[DH:DH + 1, :, :])
                t_d4 = denp.tile([4, TC], BF16, tag="d4", name=f"d4{hp}")
                nc.sync.dma_start(t_d4[:], sden[hp][:])
                t_l4 = denp.tile([4, TC], F32, tag="l4", name=f"l4{hp}")
                nc.scalar.activation(t_l4[:], t_d4[:], AF.Ln)
                t_r4 = denp.tile([4, TC], BF16, tag="r4", name=f"r4{hp}")
                nc.scalar.activation(t_r4[:], t_l4[:], AF.Exp, scale=-1.0)
                nc.sync.dma_start(srec[hp][:], t_r4[:])
                for dn in range(NCH):
                    sl_ = slice(dn * TC, (dn + 1) * TC)
                    for hh in range(2):
                        r = 2 * dn + hh
                        rb = rbp.tile([DH, TC], BF16, tag="rb",
                                      name=f"rb{hp}_{r}")
                        nc.sync.dma_start(
                            rb[:],
                            srec[hp][r:r + 1, :].broadcast_to((DH, TC)))
                        if hh == 0:
                            nc.gpsimd.tensor_mul(t_ctxn[0:DH, hp, sl_],
                                                 allcs[0:DH, r, :], rb[:])
                        else:
                            cs2 = rbp.tile([DH, TC], FP8, tag="cs",
                                           name=f"cs2{hp}_{dn}")
                            nc.gpsimd.tensor_mul(cs2[:],
                                                 allcs[0:DH, r, :], rb[:])
                            nc.gpsimd.dma_start(t_ctxn[DH:P, hp, sl_],
                                                cs2[:])

            def build_proj_steps(hp):
                """Prefetch wj DMAs for head-pair hp; return (qb, kb,
                [q_steps, k_steps]) of closures (DR matmuls + bias op) to
                interleave into the previous hp's attention loop."""
                qb = qks.tile([P, SL], BF16, tag="qk", name=f"qb{hp}")
                kb = qks.tile([P, SL], BF16, tag="qk", name=f"kb{hp}")
                halves = []
                for blk, dstt, s1 in ((hp, qb, 1.0 / (WS_QKV * 8.0)),
                                      (8 + hp, kb, 1.0 / WS_QKV)):
                    wj = wstream.tile([P, NT, P], FP8, tag="wqk",
                                      name=f"wj{blk}")
                    [nc.sync, nc.gpsimd][blk % 2].dma_start(
                        wj[:], wqkv[:, blk * P:(blk + 1) * P]
                        .rearrange("(n p) m -> p n m", p=P))
                    steps = []
                    for c in range(NCH):
                        box = {}

                        def mk_mm(kp, c=c, blk=blk, wj=wj, box=box):
                            def f():
                                if kp == 0:
                                    box["ps"] = acc.tile(
                                        [P, TC], F32, tag="acc",
                                        name=f"qk{blk}_{c}")
                                nc.tensor.matmul(
                                    box["ps"][:], wj[:, 2 * kp:2 * kp + 2, :],
                                    t_y[:, 2 * kp:2 * kp + 2,
                                        c * TC:(c + 1) * TC],
                                    start=(kp == 0), stop=(kp == NT // 2 - 1),
                                    perf_mode=DRM)
                            return f

                        for kp in range(NT // 2):
                            steps.append(mk_mm(kp))

                        def mk_bias(c=c, blk=blk, dstt=dstt, box=box, s1=s1):
                            def f():
                                nc.vector.tensor_scalar(
                                    dstt[:, c * TC:(c + 1) * TC],
                                    box["ps"][:], s1,
                                    c_bqk[:, blk:blk + 1],
                                    ALU.mult, ALU.add)
                            return f

                        steps.append(mk_bias())
                    halves.append(steps)
                return qb, kb, halves

            # hp7's attention loop hides the k-pair 0..2 partial chains of
            # out-proj blocks j=0/1; partials drain (pre-scaled, +resid)
            # to SBUF and a final (6,7)-pair pass finishes them.
            out_wos = {}
            out_part = {}
            xrs = {}

            def build_out_steps(j):
                wo = wstream.tile([P, NT, P], FP8, tag="wqk", name=f"wo{j}")
                nc.sync.dma_start(
                    wo[:], wout[:, j * P:(j + 1) * P]
                    .rearrange("(n p) m -> p n m", p=P))
                out_wos[j] = wo
                xr = xrs[j]
                steps = []
                for c in range(NCH):
                    box = {}

                    def mk_mm(kp, c=c, j=j, wo=wo, box=box):
                        def f():
                            if kp == 0:
                                box["ps"] = acc.tile(
                                    [P, TC], F32, tag="acc",
                                    name=f"opp{j}_{c}")
                            nc.tensor.matmul(
                                box["ps"][:], wo[:, 2 * kp:2 * kp + 2, :],
                                t_ctxn[:, 2 * kp:2 * kp + 2,
                                       c * TC:(c + 1) * TC],
                                start=(kp == 0), stop=(kp == 2),
                                perf_mode=DRM)
                        return f

                    for kp in range(3):
                        steps.append(mk_mm(kp))

                    def mk_drain(c=c, j=j, box=box, xr=xr):
                        def f():
                            # partial*scale + (residual + bias): the final
                            # pair pass then adds its own scaled psum.
                            pp = stream.tile([P, TC], F32, tag="opart",
                                             bufs=8, name=f"opart{j}_{c}")
                            nc.vector.scalar_tensor_tensor(
                                pp[:], box["ps"][:], OUT_SCALE,
                                xr[:, c * TC:(c + 1) * TC],
                                ALU.mult, ALU.add)
                            out_part[(j, c)] = pp
                        return f

                    steps.append(mk_drain())
                return steps

            def stage_xr():
                for j in range(NT):
                    xr = stream.tile([P, SL], BF16, tag="xr", bufs=8,
                                     name=f"xr{j}")
                    [nc.sync, nc.gpsimd][j % 2].dma_start(
                        xr[:], xt[j * P:(j + 1) * P, :])
                    xrs[j] = xr

            # hp0 qk proj first so the attention loop starts as soon
            # as y lands; V-proj (vaug is only needed by the first ctx,
            # ~10us into the loop) fills the PE behind it.
            qb_cur, kb_cur, halves0 = build_proj_steps(0)
            for st in halves0[0] + halves0[1]:
                st()
            # ============ Phase 2: V projection (token layout, DR) ======
            for i in range(NT):
                dst = t_vaug[:, i, :].rearrange("p (h c) -> p h c", c=VST)
                nc.vector.memset(dst[:, :, DH:DH + 1], ONES_COL)
                for n in range(NCH):
                    ps_wn = acc.tile([P, TC], F32, tag="acc",
                                     name=f"vps{i}_{n}")
                    for kp in range(NT // 2):
                        nc.tensor.matmul(
                            ps_wn[:],
                            t_y[:, 2 * kp:2 * kp + 2, i * P:(i + 1) * P],
                            wv_big[:, 2 * kp:2 * kp + 2, n * TC:(n + 1) * TC],
                            start=(kp == 0), stop=(kp == NT // 2 - 1),
                            perf_mode=DRM)
                    nc.scalar.activation(
                        dst[:, 8 * n:8 * (n + 1), 0:DH],
                        ps_wn[:].rearrange("p (h c) -> p h c", c=DH),
                        AF.Copy, scale=1.0 / WS_QKV)

            pending = None
            pend_ctx = []
            norm_q = []
            for hp in range(NHEAD // 2):
                if hp == 4:
                    stage_xr()
                if hp < NHEAD // 2 - 1:
                    qb_nxt, kb_nxt, halves_nxt = build_proj_steps(hp + 1)
                else:
                    qb_nxt = kb_nxt = None
                    halves_nxt = [build_out_steps(0), build_out_steps(1)]

                qb, kb = qb_cur, kb_cur
                t_cs = csp.tile([DH + 1, 4, TC], BF16, tag="cs",
                                name=f"cs{hp}")
                ctx_ps = [[None] * NCH for _ in range(2)]

                def emit_ctx(p, n, ptp, hp=hp, ctx_ps=ctx_ps, t_cs=t_cs):
                    for hh in range(2):
                        if p == 0:
                            ctx_ps[hh][n] = acc.tile(
                                [DH + 1, TC], F32, tag="acc",
                                name=f"ctx{hp}_{hh}_{n}")
                        h = 2 * hp + hh
                        nc.tensor.matmul(
                            ctx_ps[hh][n][:],
                            t_vaug[:, 2 * p:2 * p + 2,
                                   h * VST:h * VST + DH + 1],
                            ptp[:, :, hh * TC:(hh + 1) * TC],
                            start=(p == 0), stop=(p == 3),
                            perf_mode=DRM)
                    if p == 3:
                        for hh in range(2):
                            nc.vector.tensor_copy(
                                t_cs[:, 2 * n + hh, :], ctx_ps[hh][n][:])
                        if n == NCH - 1 and hp < NHEAD // 2 - 1:
                            norm_q.append((hp, t_cs))

                for n in range(NCH):
                    inj = list(halves_nxt[n])
                    ptp = None
                    for jt in range(NT):
                        sl_ = slice(n * TC, (n + 1) * TC)
                        # non-critical PE work first so the WAR wait on the
                        # wide slot overlaps it
                        if jt % 2 == 1 and len(pend_ctx) > 2:
                            e = pend_ctx.pop(0)
                            e[0](*e[1:])
                        if norm_q:
                            normalize(*norm_q.pop(0))
                        for _ in range(2):
                            if inj:
                                inj.pop(0)()
                        ps_s = wide.tile([P, SL], F32, tag="wide",
                                         name=f"s{hp}_{jt}_{n}")
                        nc.tensor.matmul(
                            ps_s[:, 0:TC],
                            kb[0:DH, jt * P:(jt + 1) * P],
                            qb[0:DH, sl_],
                            start=True, stop=True, tile_position=(0, 0))
                        nc.tensor.matmul(
                            ps_s[:, TC:2 * TC],
                            kb[DH:P, jt * P:(jt + 1) * P],
                            qb[DH:P, sl_],
                            start=True, stop=True, tile_position=(DH, 0))
                        if jt % 2 == 0:
                            ptp = pts.tile([P, 2, SL], FP8, tag="pt",
                                           name=f"pt{hp}_{n}_{jt // 2}")
                        dst = ptp[:, jt % 2, :]
                        if jt in SJT:
                            nc.vector.tensor_scalar(
                                dst.bitcast(U8), ps_s[:], LOG2E8,
                                c_mbs[:, jt:jt + 1], ALU.mult, ALU.add)
                        else:
                            nc.scalar.activation(dst, ps_s[:], AF.Exp,
                                                 bias=c_mb[:, jt:jt + 1])
                        if jt % 2 == 1:
                            pend_ctx.append((emit_ctx, jt // 2, n, ptp))
                    for st in inj:
                        st()
                if hp == NHEAD // 2 - 1:
                    for e in pend_ctx:
                        e[0](*e[1:])
                    pend_ctx = []
                pending = (hp, t_cs)
                qb_cur, kb_cur = qb_nxt, kb_nxt
            # last hp: low-latency in-SBUF normalize (no DRAM hops)
            fhp, fcs = pending
            for dn in range(NCH):
                sl_ = slice(dn * TC, (dn + 1) * TC)
                ps_rb = wide.tile([P, SL], F32, tag="wide", name=f"rbps{dn}")
                for hh in range(2):
                    r = 2 * dn + hh
                    row = fcs[DH:DH + 1, r, :]
                    nc.scalar.activation(row, row, AF.Ln)
                    nc.scalar.activation(row, row, AF.Exp, scale=-1.0)
                    nc.tensor.matmul(ps_rb[0:DH, hh * TC:(hh + 1) * TC],
                                     c_or65[DH:DH + 1, 0:DH], row,
                                     start=True, stop=True,
                                     tile_position=(DH, 0))
                for hh in range(2):
                    r = 2 * dn + hh
                    rb = rbp.tile([DH, TC], BF16, tag="rb",
                                  name=f"rbf{hh}_{dn}")
                    nc.vector.tensor_copy(
                        rb[:], ps_rb[0:DH, hh * TC:(hh + 1) * TC])
                    if hh == 0:
                        nc.gpsimd.tensor_mul(t_ctxn[0:DH, fhp, sl_],
                                             fcs[0:DH, r, :], rb[:])
                    else:
                        cs2 = rbp.tile([DH, TC], FP8, tag="cs",
                                       name=f"cs2f{dn}")
                        nc.gpsimd.tensor_mul(cs2[:], fcs[0:DH, r, :], rb[:])
                        nc.gpsimd.dma_start(t_ctxn[DH:P, fhp, sl_], cs2[:])

            # ================= Phase 5: out-proj + residual =============
            wos = {j: out_wos[j] for j in out_wos}
            for j in range(2, NT):
                wo = wstream.tile([P, NT, P], FP8, tag="wqk", name=f"wo{j}")
                nc.sync.dma_start(
                    wo[:], wout[:, j * P:(j + 1) * P]
                    .rearrange("(n p) m -> p n m", p=P))
                wos[j] = wo
            qi = 0
            for j in [2, 3, 4, 5, 0, 1, 6, 7]:
                wo = wos[j]
                for n in range(NCH):
                    sl_ = slice(n * TC, (n + 1) * TC)
                    ot = stream.tile([P, TC], BF16, tag="ot", bufs=4,
                                     name=f"ot{j}_{n}")
                    if j < 2:
                        # finish the hp7-hidden partial: (6,7) pair + part
                        ps_o = acc.tile([P, TC], F32, tag="acc",
                                        name=f"opf{j}_{n}")
                        nc.tensor.matmul(ps_o[:], wo[:, NT - 2:NT, :],
                                         t_ctxn[:, NT - 2:NT, sl_],
                                         start=True, stop=True,
                                         perf_mode=DRM)
                        nc.vector.scalar_tensor_tensor(
                            ot[:], ps_o[:], OUT_SCALE,
                            out_part[(j, n)][:], ALU.mult, ALU.add)
                    else:
                        ps_o = acc.tile([P, TC], F32, tag="acc",
                                        name=f"ops{j}_{n}")
                        for kp in range(NT // 2):
                            nc.tensor.matmul(
                                ps_o[:], wo[:, 2 * kp:2 * kp + 2, :],
                                t_ctxn[:, 2 * kp:2 * kp + 2, sl_],
                                start=(kp == 0), stop=(kp == NT // 2 - 1),
                                perf_mode=DRM)
                        nc.vector.scalar_tensor_tensor(
                            ot[:], ps_o[:], OUT_SCALE,
                            xrs[j][:, sl_], ALU.mult, ALU.add)
                    # split the store across queues; last tiles 4-way
                    nsplit = 4 if j >= 6 else 2
                    w = TC // nsplit
                    for sp in range(nsplit):
                        eng = [nc.sync, nc.gpsimd, nc.scalar][qi % 3]
                        qi += 1
                        eng.dma_start(
                            out[j * P:(j + 1) * P,
                                n * TC + sp * w:n * TC + (sp + 1) * w],
                            ot[:, sp * w:(sp + 1) * w])

    if hoist:
        _hoist_waits(nc)
    return nc


_NC_CACHE = None


def _get_nc():
    global _NC_CACHE
    if _NC_CACHE is None:
        _NC_CACHE = _build_nc()
    return _NC_CACHE


def _prep_in_maps(hidden_states, encoder_padding_mask, in_proj_weight,
                  in_proj_bias, out_proj_weight, out_proj_bias,
                  norm_weight, norm_bias):
    import ml_dtypes
    f = np.float32
    bf = ml_dtypes.bfloat16
    f8 = ml_dtypes.float8_e4m3
    w2 = np.asarray(in_proj_weight, dtype=f).reshape(3 * HS, HS).copy()
    b2 = np.asarray(in_proj_bias, dtype=f).reshape(3 * HS).copy()
    # fold the LN affine (w, b) into the fused projection: W*(y*w+b)+bias
    # == (W*diag(w))*y + (bias + W@b)
    nw = np.asarray(norm_weight, dtype=f).reshape(HS)
    nb = np.asarray(norm_bias, dtype=f).reshape(HS)
    b2 = b2 + w2 @ nb
    w2 = w2 * nw[None, :]
    # 1/sqrt(dh) on the q bias only; the weight part is applied at the
    # psum->qb copy (s1 = 1/(WS_QKV*8)).
    b2[0:HS] *= f(1.0 / np.sqrt(DH))
    wqkv = np.ascontiguousarray(w2.T * WS_QKV).astype(f8)     # [d, 3HS]
    wout = np.ascontiguousarray(
        np.asarray(out_proj_weight, dtype=f).T * WS_OUT).astype(f8)
    bqk = np.ascontiguousarray(b2[:2 * HS].reshape(16, P).T)
    wo_f = np.asarray(out_proj_weight, dtype=f)
    bo = np.asarray(out_proj_bias, dtype=f).reshape(HS) + wo_f @ b2[2 * HS:]
    onesr = np.ones((1, P), dtype=bf)
    epsr = np.full((1, 1), LN_EPS, f)
    shared = dict(wqkv=wqkv, wout=wout, bqk=bqk,
                  onesr=onesr, epsr=epsr)

    hs = np.asarray(hidden_states, dtype=f)
    mask = np.asarray(encoder_padding_mask)
    in_maps = []
    for c in range(BS):
        mbc = (mask[c].astype(f) * f(MASK_NEG)).reshape(NT, P).T
        mbc = np.ascontiguousarray(mbc)
        mbsc = np.ascontiguousarray(
            f(SC_C) + f(LOG2E8) * mbc).astype(f)
        xtc = np.ascontiguousarray(hs[c].T) + bo[:, None]
        in_maps.append(dict(
            xt=xtc.astype(bf),
            x8d=np.ascontiguousarray(hs[c].T).astype(f8),
            mb=mbc,
            mbs=mbsc,
            **shared,
        ))
    return in_maps


def _run(in_maps, trace=False):
    nc = _get_nc()
    return run_bass_kernel_spmd(nc, in_maps, list(range(BS)), trace=trace)


def kernel(**inputs):
    in_maps = _prep_in_maps(**inputs)
    res = _run(in_maps, trace=False)
    outs = [res.results[c]["out"].T for c in range(BS)]
    return np.stack(outs, axis=0).astype(np.float32)


def kernel_traced(**inputs):
    in_maps = _prep_in_maps(**inputs)
    res = _run(in_maps, trace=True)
    outs = [res.results[c]["out"].T for c in range(BS)]
    return np.stack(outs, axis=0).astype(np.float32), res.exec_time_ns


# revision 26
# speedup vs baseline: 1.0203x; 1.0203x over previous
"""TRN2 Bass kernel for nn_MultiHeadAttention_50835232916148 (fp8 rewrite).

Pre-LN MHA block (HS=1024, 16 heads, bs=8, sl=1024), data-parallel over
batch across 8 NeuronCores (bs=1 per core, no collectives).

v2 design (vs the bf16 baseline at ~284us):
- All big matmuls (QKV proj, V proj, ctx, out-proj, LN stats) run as fp8e4
  DoubleRow pairs: one instruction contracts 2 k-tiles (256 deep) in the
  time bf16 streams one, measured 2.37x per-instruction on HW.  Weights are
  host-scaled x32 (x16 for out-proj) into e4m3's normal range; the inverse
  scale rides the psum->SBUF bias ops as a free immediate multiplier.
- Scores stay bf16 (64-deep contraction is column-bound; fp8 DR measured
  slower in that shape).
- The exp stream is split: 5 of 8 key-tiles per (hp, chunk) run on the ACT
  engine (exact exp, fp8 out), 3 run on DVE as a one-op "Schraudolph-fp8":
  bits = RNE(score*8/ln2 + 55.54 + mask_bias) computed by tensor_scalar
  into uint8 = the e4m3 bit pattern of ~exp(score) (3% rms).  Negative
  saturation gives masked keys exactly 0.
- Softmax denominator rides the ctx matmul as a 65th vaug column of 1/256
  (so the reciprocal lands in e4m3/bf16 sweet spot); recip is one DVE
  reciprocal per head-pair on DRAM-gathered rows; normalize muls run on
  the otherwise-idle Pool engine (SBUF-only: GPSIMD cannot touch PSUM).
- out_proj bias + residual are host-folded into the fp32 xt tensor, so the
  out-proj tail is one scalar_tensor_tensor per tile.

Per-core dataflow ([feature, token] transposed activations):
  x8 (fp8, from host) --fp8-DR ones-matmul stats--> istd, b2
  y = x8*bcast(istd) + bcast(b2)                      [d,t] fp8
  vaug[t, h*65:(h+1)*65] = [y.T@Wv + bv | 1/256]      token-major fp8
  per head-pair hp: qb,kb = WqkT.T @ y + b (DR, interleaved into hp-1)
    per (n, jt): scoresT = [kbA^T qbA | kbB^T qbB]    wide 2-bank PSUM
                 pt[jt&1] = exp-ish(scoresT + mask)   ACT or DVE-schraudolph
                 per pair p: ctx_aug += vaug[2p:2p+2].T @ pt-pair (fp8 DR)
    recip = DVE reciprocal on DRAM-gathered denom rows; rb = bcast DMA
    ctxn = cs * rb on Pool (fp8 out, head B shifted to partitions 64-127)
  outT = WoutT.T @ ctxn (DR); out = outT*2^-12 + (xT + bo)  one DVE stt
"""

import numpy as np

import concourse.bass as bass
import concourse.mybir as mybir
import concourse.tile as tile
from concourse.bass_utils import run_bass_kernel_spmd

P = 128
HS = 1024
SL = 1024
NHEAD = 16
DH = 64
BS = 8
NT = HS // P          # 8 feature/token tiles
TC = 512              # matmul free-dim chunk (fp32 PSUM bank)
NCH = SL // TC        # 2
LN_EPS = 1e-5
MASK_NEG = -1e8
F32 = mybir.dt.float32
F32R = mybir.dt.float32r
BF16 = mybir.dt.bfloat16
FP8 = mybir.dt.float8e4
U8 = mybir.dt.uint8
AF = mybir.ActivationFunctionType
ALU = mybir.AluOpType
DRM = mybir.MatmulPerfMode.DoubleRow

WS_QKV = 32.0         # host scale on wqkv (e4m3 normal range)
WS_OUT = 16.0         # host scale on wout
ONES_COL = 1.0 / 256.0  # vaug denominator column value
OUT_SCALE = 1.0 / (WS_OUT * 256.0)
LOG2E8 = 8.0 / np.log(2.0)   # schraudolph slope
SC_C = 56.0 - 0.46           # schraudolph intercept (calibrated)
SJT = (3, 7)                 # jt slots whose exp runs on DVE


def _hoist_waits(nc):
    """walrus in this env rejects >1 inline wait per instruction and ANY
    inline wait on Matmult; hoist them onto single-wait NoOps."""
    n_fixed = 0
    for _, bb in nc.bb_map.items():
        inner = bb.bb
        insts = inner.instructions
        new = []
        changed = False
        for inst in insts:
            si = getattr(inst, "sync_info", None)
            if si is not None and si.on_wait:
                keep = 0 if isinstance(inst, mybir.InstMatmult) else 1
                waits = list(si.on_wait)
                if len(waits) > keep:
                    kept = waits[-keep:] if keep else []
                    for w in waits[: len(waits) - keep]:
                        new.append(
                            mybir.InstNoOp(
                                name=nc.get_next_instruction_name(),
                                sync_info=mybir.SyncInfo(on_wait=[w], on_update=[]),
                                bass_nofuse=True,
                                engine=inst.engine,
                            )
                        )
                    inst.sync_info = mybir.SyncInfo(
                        on_wait=kept, on_update=list(si.on_update)
                    )
                    n_fixed += 1
                    changed = True
            new.append(inst)
        if changed:
            inner.instructions = new
    return n_fixed


def _build_nc(hoist=True):
    nc = bass.Bass()

    xt = nc.dram_tensor("xt", [HS, SL], BF16, kind="ExternalInput")
    x8d = nc.dram_tensor("x8d", [HS, SL], FP8, kind="ExternalInput")
    wqkv = nc.dram_tensor("wqkv", [HS, 3 * HS], FP8, kind="ExternalInput")
    wout = nc.dram_tensor("wout", [HS, HS], FP8, kind="ExternalInput")
    bqk = nc.dram_tensor("bqk", [P, 16], F32, kind="ExternalInput")
    mb = nc.dram_tensor("mb", [P, NT], F32, kind="ExternalInput")
    mbs = nc.dram_tensor("mbs", [P, NT], F32, kind="ExternalInput")
    onesr = nc.dram_tensor("onesr", [1, P], BF16, kind="ExternalInput")
    epsr = nc.dram_tensor("epsr", [1, 1], F32, kind="ExternalInput")
    out = nc.dram_tensor("out", [HS, SL], BF16, kind="ExternalOutput")
    sden = [nc.dram_tensor(f"sden{h}", [4, TC], BF16, kind="Internal")
            for h in range(NHEAD // 2)]
    srec = [nc.dram_tensor(f"srec{h}", [4, TC], BF16, kind="Internal")
            for h in range(NHEAD // 2)]

    with tile.TileContext(nc) as tc, nc.allow_low_precision(
            reason="fp8 matmuls; tolerance is 2e-2 and residual is fp32"):
        with (
            tc.tile_pool(name="big", bufs=1) as big,
            tc.tile_pool(name="wstream", bufs=6) as wstream,
            tc.tile_pool(name="scratch", bufs=2) as scratch,
            tc.tile_pool(name="qks", bufs=4) as qks,
            tc.tile_pool(name="pts", bufs=4) as pts,
            tc.tile_pool(name="stream", bufs=3) as stream,
            tc.tile_pool(name="vecs", bufs=1) as vecs,
            tc.tile_pool(name="csp", bufs=2) as csp,
            tc.tile_pool(name="rbp", bufs=4) as rbp,
            tc.tile_pool(name="denp", bufs=2) as denp,
            tc.tile_pool(name="consts", bufs=1) as consts,
            tc.tile_pool(name="wide", bufs=2, space="PSUM") as wide,
            tc.tile_pool(name="acc", bufs=4, space="PSUM") as acc,
        ):
            # ---- big activation tiles ----
            t_xb = big.tile([P, NT, SL], FP8, tag="xb")
            t_x16 = big.tile([P, NT, SL], BF16, tag="x16")
            t_sq = big.tile([P, NT, SL], BF16, tag="sq")
            t_y = big.tile([P, NT, SL], FP8, tag="y")
            VST = DH + 2  # head stride in vaug (even, for fp8 dual-load)
            t_vaug = big.tile([P, NT, NHEAD * VST], FP8, tag="vaug")
            t_ctxn = big.tile([P, NT, SL], FP8, tag="ctxn")
            wv_big = big.tile([P, NT, HS], FP8, tag="wv")

            # ACT table warmup first: memset-fed exp has no DMA deps, so
            # the ~2.7us table load runs during the framework preamble
            t_warm = vecs.tile([1, 1], F32, tag="warm")
            nc.vector.memset(t_warm[:], 1.0)
            nc.scalar.activation(t_warm[:], t_warm[:], AF.Exp)
            # PE clock warmup (HAM): memset-fed, no DMA deps
            c_oneb = consts.tile([P, 1], BF16, tag="oneb")
            nc.vector.memset(c_oneb[:], 1.0)
            ps_warm = acc.tile([1, TC], F32, tag="acc", name="warmps")
            for _ in range(60):
                nc.tensor.matmul(ps_warm[:, 0:1], c_oneb[:], c_oneb[:],
                                 start=True, stop=True)

            # ---- input DMA staging ----
            # first x8 tiles split across queues so stats start early
            for i in range(NT):
                nc.sync.dma_start(t_xb[:, i, 0:TC],
                                  x8d[i * P:(i + 1) * P, 0:TC])
                nc.gpsimd.dma_start(t_xb[:, i, TC:SL],
                                    x8d[i * P:(i + 1) * P, TC:SL])
            for k in range(NT):
                [nc.sync, nc.gpsimd][k % 2].dma_start(
                    wv_big[:, k, :], wqkv[k * P:(k + 1) * P, 2 * HS:3 * HS])
            c_bqk = consts.tile([P, 16], F32, tag="bqk")
            nc.sync.dma_start(c_bqk[:], bqk[:])
            c_mb = consts.tile([P, NT], F32, tag="mb")
            nc.sync.dma_start(c_mb[:], mb[:])
            c_mbs = consts.tile([P, NT], F32, tag="mbs")
            nc.gpsimd.dma_start(c_mbs[:], mbs[:])
            c_or = consts.tile([1, P], BF16, tag="onesr")
            nc.gpsimd.dma_start(c_or[:], onesr[:])
            c_or65 = consts.tile([DH + 1, P], BF16, tag="or65")
            nc.gpsimd.dma_start(c_or65[DH:DH + 1, :], onesr[:])
            c_eps = consts.tile([1, 1], F32, tag="eps")
            nc.sync.dma_start(c_eps[:], epsr[:])

            # ================= Phase 1: LayerNorm =================
            # ACT converts fp8 x -> bf16 (full speed there); sq/stats/y all
            # run in fast bf16 paths; y goes back to fp8 via ACT copies.
            for i in range(NT):
                nc.scalar.activation(t_x16[:, i, :], t_xb[:, i, :], AF.Copy)
            st_m = wide.tile([1, SL], F32, tag="wide", name="stm")
            st_s = wide.tile([1, SL], F32, tag="wide", name="sts")
            for i in range(NT):
                nc.vector.tensor_mul(t_sq[:, i, :], t_x16[:, i, :],
                                     t_x16[:, i, :])
            for i in range(NT):
                for n in range(NCH):
                    sl_ = slice(n * TC, (n + 1) * TC)
                    nc.tensor.matmul(st_m[:, sl_], c_oneb[:],
                                     t_x16[:, i, sl_],
                                     start=(i == 0), stop=(i == NT - 1))
                    nc.tensor.matmul(st_s[:, sl_], c_oneb[:],
                                     t_sq[:, i, sl_],
                                     start=(i == 0), stop=(i == NT - 1))
            # LN tail chunked per 512-column half (baseline-proven chain)
            v_mean = vecs.tile([1, SL], F32, tag="mean")
            v_msq = vecs.tile([1, SL], F32, tag="msq")
            v_tmp = vecs.tile([1, SL], F32, tag="tmp")
            v_lnv = vecs.tile([1, SL], F32, tag="lnv")
            v_istd = vecs.tile([1, SL], BF16, tag="istd")
            v_b2 = vecs.tile([1, SL], BF16, tag="b2")
            t_A = scratch.tile([P, SL], BF16, tag="ab", name="tA")
            t_B = scratch.tile([P, SL], BF16, tag="ab", name="tB")
            for c in range(NCH):
                cl = slice(c * TC, (c + 1) * TC)
                nc.scalar.activation(v_msq[:, cl], st_s[:, cl], AF.Copy,
                                     scale=1.0 / HS)
                nc.scalar.activation(v_mean[:, cl], st_m[:, cl], AF.Copy,
                                     scale=1.0 / HS)
                nc.vector.tensor_mul(v_tmp[:, cl], v_mean[:, cl],
                                     v_mean[:, cl])
                nc.vector.tensor_sub(v_msq[:, cl], v_msq[:, cl],
                                     v_tmp[:, cl])   # -> var
                nc.scalar.activation(v_lnv[:, cl], v_msq[:, cl], AF.Ln,
                                     bias=c_eps[:])
                nc.scalar.activation(v_istd[:, cl], v_lnv[:, cl], AF.Exp,
                                     scale=-0.5)
                nc.vector.scalar_tensor_tensor(v_b2[:, cl], v_mean[:, cl],
                                               -1.0, v_istd[:, cl],
                                               ALU.mult, ALU.mult)
                pA = acc.tile([P, TC], F32, tag="acc", name=f"pA{c}")
                nc.tensor.matmul(pA[:], c_or[:], v_istd[:, cl],
                                 start=True, stop=True)
                nc.vector.tensor_copy(t_A[:, cl], pA[:])
                pB = acc.tile([P, TC], F32, tag="acc", name=f"pB{c}")
                nc.tensor.matmul(pB[:], c_or[:], v_b2[:, cl],
                                 start=True, stop=True)
                nc.vector.tensor_copy(t_B[:, cl], pB[:])
            # y: bf16 affine on DVE, fp8 via ACT copy, i-major
            for i in range(NT):
                yb = stream.tile([P, SL], BF16, tag="yb", bufs=3,
                                 name=f"yb{i}")
                for c in range(NCH):
                    cl = slice(c * TC, (c + 1) * TC)
                    t1 = stream.tile([P, TC], BF16, tag="t1v", bufs=3,
                                     name=f"yt{i}_{c}")
                    nc.vector.tensor_mul(t1[:], t_x16[:, i, cl], t_A[:, cl])
                    nc.vector.tensor_add(yb[:, cl], t1[:], t_B[:, cl])
                nc.scalar.activation(t_y[:, i, :], yb[:], AF.Copy)

            # ========= Phase 3+4: per head-pair QK proj + attention =====
            def normalize(hp, allcs):
                """allcs = [65, 4, TC] bf16; col r = 2*dn+hh.  Gather denom
                rows through DRAM, one DVE reciprocal, bcast back, Pool
                muls into ctxn (fp8)."""
                nc.sync.dma_start(sden[hp][:], allcs[DH:DH + 1, :, :])
                t_d4 = denp.tile([4, TC], BF16, tag="d4", name=f"d4{hp}")
                nc.sync.dma_start(t_d4[:], sden[hp][:])
                t_l4 = denp.tile([4, TC], F32, tag="l4", name=f"l4{hp}")
                nc.scalar.activation(t_l4[:], t_d4[:], AF.Ln)
                t_r4 = denp.tile([4, TC], BF16, tag="r4", name=f"r4{hp}")
                nc.scalar.activation(t_r4[:], t_l4[:], AF.Exp, scale=-1.0)
                nc.sync.dma_start(srec[hp][:], t_r4[:])
                for dn in range(NCH):
                    sl_ = slice(dn * TC, (dn + 1) * TC)
                    for hh in range(2):
                        r = 2 * dn + hh
                        rb = rbp.tile([DH, TC], BF16, tag="rb",
                                      name=f"rb{hp}_{r}")
                        nc.sync.dma_start(
                            rb[:],
                            srec[hp][r:r + 1, :].broadcast_to((DH, TC)))
                        if hh == 0:
                            nc.gpsimd.tensor_mul(t_ctxn[0:DH, hp, sl_],
                                                 allcs[0:DH, r, :], rb[:])
                        else:
                            cs2 = rbp.tile([DH, TC], FP8, tag="cs",
                                           name=f"cs2{hp}_{dn}")
                            nc.gpsimd.tensor_mul(cs2[:],
                                                 allcs[0:DH, r, :], rb[:])
                            nc.gpsimd.dma_start(t_ctxn[DH:P, hp, sl_],
                                                cs2[:])

            def build_proj_steps(hp):
                """Prefetch wj DMAs for head-pair hp; return (qb, kb,
                [q_steps, k_steps]) of closures (DR matmuls + bias op) to
                interleave into the previous hp's attention loop."""
                qb = qks.tile([P, SL], BF16, tag="qk", name=f"qb{hp}")
                kb = qks.tile([P, SL], BF16, tag="qk", name=f"kb{hp}")
                halves = []
                for blk, dstt, s1 in ((hp, qb, 1.0 / (WS_QKV * 8.0)),
                                      (8 + hp, kb, 1.0 / WS_QKV)):
                    wj = wstream.tile([P, NT, P], FP8, tag="wqk",
                                      name=f"wj{blk}")
                    [nc.sync, nc.gpsimd][blk % 2].dma_start(
                        wj[:], wqkv[:, blk * P:(blk + 1) * P]
                        .rearrange("(n p) m -> p n m", p=P))
                    steps = []
                    for c in range(NCH):
                        box = {}

                        def mk_mm(kp, c=c, blk=blk, wj=wj, box=box):
                            def f():
                                if kp == 0:
                                    box["ps"] = acc.tile(
                                        [P, TC], F32, tag="acc",
                                        name=f"qk{blk}_{c}")
                                nc.tensor.matmul(
                                    box["ps"][:], wj[:, 2 * kp:2 * kp + 2, :],
                                    t_y[:, 2 * kp:2 * kp + 2,
                                        c * TC:(c + 1) * TC],
                                    start=(kp == 0), stop=(kp == NT // 2 - 1),
                                    perf_mode=DRM)
                            return f

                        for kp in range(NT // 2):
                            steps.append(mk_mm(kp))

                        def mk_bias(c=c, blk=blk, dstt=dstt, box=box, s1=s1):
                            def f():
                                nc.vector.tensor_scalar(
                                    dstt[:, c * TC:(c + 1) * TC],
                                    box["ps"][:], s1,
                                    c_bqk[:, blk:blk + 1],
                                    ALU.mult, ALU.add)
                            return f

                        steps.append(mk_bias())
                    halves.append(steps)
                return qb, kb, halves

            # hp7's attention loop hides the k-pair 0..2 partial chains of
            # out-proj blocks j=0/1; partials drain (pre-scaled, +resid)
            # to SBUF and a final (6,7)-pair pass finishes them.
            out_wos = {}
            out_part = {}
            xrs = {}

            def build_out_steps(j):
                wo = wstream.tile([P, NT, P], FP8, tag="wqk", name=f"wo{j}")
                nc.sync.dma_start(
                    wo[:], wout[:, j * P:(j + 1) * P]
                    .rearrange("(n p) m -> p n m", p=P))
                out_wos[j] = wo
                xr = xrs[j]
                steps = []
                for c in range(NCH):
                    box = {}

                    def mk_mm(kp, c=c, j=j, wo=wo, box=box):
                        def f():
                            if kp == 0:
                                box["ps"] = acc.tile(
                                    [P, TC], F32, tag="acc",
                                    name=f"opp{j}_{c}")
                            nc.tensor.matmul(
                                box["ps"][:], wo[:, 2 * kp:2 * kp + 2, :],
                                t_ctxn[:, 2 * kp:2 * kp + 2,
                                       c * TC:(c + 1) * TC],
                                start=(kp == 0), stop=(kp == 2),
                                perf_mode=DRM)
                        return f

                    for kp in range(3):
                        steps.append(mk_mm(kp))

                    def mk_drain(c=c, j=j, box=box, xr=xr):
                        def f():
                            # partial*scale + (residual + bias): the final
                            # pair pass then adds its own scaled psum.
                            pp = stream.tile([P, TC], F32, tag="opart",
                                             bufs=8, name=f"opart{j}_{c}")
                            nc.vector.scalar_tensor_tensor(
                                pp[:], box["ps"][:], OUT_SCALE,
                                xr[:, c * TC:(c + 1) * TC],
                                ALU.mult, ALU.add)
                            out_part[(j, c)] = pp
                        return f

                    steps.append(mk_drain())
                return steps

            def stage_xr():
                for j in range(NT):
                    xr = stream.tile([P, SL], BF16, tag="xr", bufs=8,
                                     name=f"xr{j}")
                    [nc.sync, nc.gpsimd][j % 2].dma_start(
                        xr[:], xt[j * P:(j + 1) * P, :])
                    xrs[j] = xr

            # hp0 qk proj first so the attention loop starts as soon
            # as y lands; V-proj (vaug is only needed by the first ctx,
            # ~10us into the loop) fills the PE behind it.
            qb_cur, kb_cur, halves0 = build_proj_steps(0)
            for st in halves0[0] + halves0[1]:
                st()
            # ============ Phase 2: V projection (token layout, DR) ======
            for i in range(NT):
                dst = t_vaug[:, i, :].rearrange("p (h c) -> p h c", c=VST)
                nc.vector.memset(dst[:, :, DH:DH + 1], ONES_COL)
                for n in range(NCH):
                    ps_wn = acc.tile([P, TC], F32, tag="acc",
                                     name=f"vps{i}_{n}")
                    for kp in range(NT // 2):
                        nc.tensor.matmul(
                            ps_wn[:],
                            t_y[:, 2 * kp:2 * kp + 2, i * P:(i + 1) * P],
                            wv_big[:, 2 * kp:2 * kp + 2, n * TC:(n + 1) * TC],
                            start=(kp == 0), stop=(kp == NT // 2 - 1),
                            perf_mode=DRM)
                    nc.scalar.activation(
                        dst[:, 8 * n:8 * (n + 1), 0:DH],
                        ps_wn[:].rearrange("p (h c) -> p h c", c=DH),
                        AF.Copy, scale=1.0 / WS_QKV)

            pending = None
            for hp in range(NHEAD // 2):
                if hp == 4:
                    stage_xr()
                if hp < NHEAD // 2 - 1:
                    qb_nxt, kb_nxt, halves_nxt = build_proj_steps(hp + 1)
                else:
                    qb_nxt = kb_nxt = None
                    halves_nxt = [build_out_steps(0), build_out_steps(1)]

                if pending is not None:
                    normalize(*pending)
                    pending = None
                qb, kb = qb_cur, kb_cur
                t_cs = csp.tile([DH + 1, 4, TC], BF16, tag="cs",
                                name=f"cs{hp}")
                ctx_ps = [[None] * NCH for _ in range(2)]

                def emit_ctx(p, n, ptp, hp=hp, ctx_ps=ctx_ps, t_cs=t_cs):
                    for hh in range(2):
                        if p == 0:
                            ctx_ps[hh][n] = acc.tile(
                                [DH + 1, TC], F32, tag="acc",
                                name=f"ctx{hp}_{hh}_{n}")
                        h = 2 * hp + hh
                        nc.tensor.matmul(
                            ctx_ps[hh][n][:],
                            t_vaug[:, 2 * p:2 * p + 2,
                                   h * VST:h * VST + DH + 1],
                            ptp[:, :, hh * TC:(hh + 1) * TC],
                            start=(p == 0), stop=(p == 3),
                            perf_mode=DRM)
                    if p == 3:
                        for hh in range(2):
                            nc.vector.tensor_copy(
                                t_cs[:, 2 * n + hh, :], ctx_ps[hh][n][:])

                pend_ctx = []
                for n in range(NCH):
                    inj = list(halves_nxt[n])
                    ptp = None
                    for jt in range(NT):
                        sl_ = slice(n * TC, (n + 1) * TC)
                        # non-critical PE work first so the WAR wait on the
                        # wide slot overlaps it
                        if jt % 2 == 1 and len(pend_ctx) > 1:
                            emit_ctx(*pend_ctx.pop(0))
                        for _ in range(2):
                            if inj:
                                inj.pop(0)()
                        ps_s = wide.tile([P, SL], F32, tag="wide",
                                         name=f"s{hp}_{jt}_{n}")
                        nc.tensor.matmul(
                            ps_s[:, 0:TC],
                            kb[0:DH, jt * P:(jt + 1) * P],
                            qb[0:DH, sl_],
                            start=True, stop=True, tile_position=(0, 0))
                        nc.tensor.matmul(
                            ps_s[:, TC:2 * TC],
                            kb[DH:P, jt * P:(jt + 1) * P],
                            qb[DH:P, sl_],
                            start=True, stop=True, tile_position=(DH, 0))
                        if jt % 2 == 0:
                            ptp = pts.tile([P, 2, SL], FP8, tag="pt",
                                           name=f"pt{hp}_{n}_{jt // 2}")
                        dst = ptp[:, jt % 2, :]
                        if jt in SJT:
                            nc.vector.tensor_scalar(
                                dst.bitcast(U8), ps_s[:], LOG2E8,
                                c_mbs[:, jt:jt + 1], ALU.mult, ALU.add)
                        else:
                            nc.scalar.activation(dst, ps_s[:], AF.Exp,
                                                 bias=c_mb[:, jt:jt + 1])
                        if jt % 2 == 1:
                            pend_ctx.append((jt // 2, n, ptp))
                    for st in inj:
                        st()
                for e in pend_ctx:
                    emit_ctx(*e)
                pending = (hp, t_cs)
                qb_cur, kb_cur = qb_nxt, kb_nxt
            # last hp: low-latency in-SBUF normalize (no DRAM hops)
            fhp, fcs = pending
            for dn in range(NCH):
                sl_ = slice(dn * TC, (dn + 1) * TC)
                ps_rb = wide.tile([P, SL], F32, tag="wide", name=f"rbps{dn}")
                for hh in range(2):
                    r = 2 * dn + hh
                    row = fcs[DH:DH + 1, r, :]
                    nc.scalar.activation(row, row, AF.Ln)
                    nc.scalar.activation(row, row, AF.Exp, scale=-1.0)
                    nc.tensor.matmul(ps_rb[0:DH, hh * TC:(hh + 1) * TC],
                                     c_or65[DH:DH + 1, 0:DH], row,
                                     start=True, stop=True,
                                     tile_position=(DH, 0))
                for hh in range(2):
                    r = 2 * dn + hh
                    rb = rbp.tile([DH, TC], BF16, tag="rb",
                                  name=f"rbf{hh}_{dn}")
                    nc.vector.tensor_copy(
                        rb[:], ps_rb[0:DH, hh * TC:(hh + 1) * TC])
                    if hh == 0:
                        nc.gpsimd.tensor_mul(t_ctxn[0:DH, fhp, sl_],
                                             fcs[0:DH, r, :], rb[:])
                    else:
                        cs2 = rbp.tile([DH, TC], FP8, tag="cs",
                                       name=f"cs2f{dn}")
                        nc.gpsimd.tensor_mul(cs2[:], fcs[0:DH, r, :], rb[:])
                        nc.gpsimd.dma_start(t_ctxn[DH:P, fhp, sl_], cs2[:])

            # ================= Phase 5: out-proj + residual =============
            wos = {j: out_wos[j] for j in out_wos}
            for j in range(2, NT):
                wo = wstream.tile([P, NT, P], FP8, tag="wqk", name=f"wo{j}")
                nc.sync.dma_start(
                    wo[:], wout[:, j * P:(j + 1) * P]
                    .rearrange("(n p) m -> p n m", p=P))
                wos[j] = wo
            qi = 0
            for j in [2, 3, 4, 5, 0, 1, 6, 7]:
                wo = wos[j]
                for n in range(NCH):
                    sl_ = slice(n * TC, (n + 1) * TC)
                    ot = stream.tile([P, TC], BF16, tag="ot", bufs=4,
                                     name=f"ot{j}_{n}")
                    if j < 2:
                        # finish the hp7-hidden partial: (6,7) pair + part
                        ps_o = acc.tile([P, TC], F32, tag="acc",
                                        name=f"opf{j}_{n}")
                        nc.tensor.matmul(ps_o[:], wo[:, NT - 2:NT, :],
                                         t_ctxn[:, NT - 2:NT, sl_],
                                         start=True, stop=True,
                                         perf_mode=DRM)
                        nc.vector.scalar_tensor_tensor(
                            ot[:], ps_o[:], OUT_SCALE,
                            out_part[(j, n)][:], ALU.mult, ALU.add)
                    else:
                        ps_o = acc.tile([P, TC], F32, tag="acc",
                                        name=f"ops{j}_{n}")
                        for kp in range(NT // 2):
                            nc.tensor.matmul(
                                ps_o[:], wo[:, 2 * kp:2 * kp + 2, :],
                                t_ctxn[:, 2 * kp:2 * kp + 2, sl_],
                                start=(kp == 0), stop=(kp == NT // 2 - 1),
                                perf_mode=DRM)
                        nc.vector.scalar_tensor_tensor(
                            ot[:], ps_o[:], OUT_SCALE,
                            xrs[j][:, sl_], ALU.mult, ALU.add)
                    # split the store across queues; last tiles 4-way
                    nsplit = 4 if j >= 6 else 2
                    w = TC // nsplit
                    for sp in range(nsplit):
                        eng = [nc.sync, nc.gpsimd, nc.scalar][qi % 3]
                        qi += 1
                        eng.dma_start(
                            out[j * P:(j + 1) * P,
                                n * TC + sp * w:n * TC + (sp + 1) * w],
                            ot[:, sp * w:(sp + 1) * w])

    if hoist:
        _hoist_waits(nc)
    return nc


_NC_CACHE = None


def _get_nc():
    global _NC_CACHE
    if _NC_CACHE is None:
        _NC_CACHE = _build_nc()
    return _NC_CACHE


def _prep_in_maps(hidden_states, encoder_padding_mask, in_proj_weight,
                  in_proj_bias, out_proj_weight, out_proj_bias,
                  norm_weight, norm_bias):
    import ml_dtypes
    f = np.float32
    bf = ml_dtypes.bfloat16
    f8 = ml_dtypes.float8_e4m3
    w2 = np.asarray(in_proj_weight, dtype=f).reshape(3 * HS, HS).copy()
    b2 = np.asarray(in_proj_bias, dtype=f).reshape(3 * HS).copy()
    # fold the LN affine (w, b) into the fused projection: W*(y*w+b)+bias
    # == (W*diag(w))*y + (bias + W@b)
    nw = np.asarray(norm_weight, dtype=f).reshape(HS)
    nb = np.asarray(norm_bias, dtype=f).reshape(HS)
    b2 = b2 + w2 @ nb
    w2 = w2 * nw[None, :]
    # 1/sqrt(dh) on the q bias only; the weight part is applied at the
    # psum->qb copy (s1 = 1/(WS_QKV*8)).
    b2[0:HS] *= f(1.0 / np.sqrt(DH))
    wqkv = np.ascontiguousarray(w2.T * WS_QKV).astype(f8)     # [d, 3HS]
    wout = np.ascontiguousarray(
        np.asarray(out_proj_weight, dtype=f).T * WS_OUT).astype(f8)
    bqk = np.ascontiguousarray(b2[:2 * HS].reshape(16, P).T)
    wo_f = np.asarray(out_proj_weight, dtype=f)
    bo = np.asarray(out_proj_bias, dtype=f).reshape(HS) + wo_f @ b2[2 * HS:]
    onesr = np.ones((1, P), dtype=bf)
    epsr = np.full((1, 1), LN_EPS, f)
    shared = dict(wqkv=wqkv, wout=wout, bqk=bqk,
                  onesr=onesr, epsr=epsr)

    hs = np.asarray(hidden_states, dtype=f)
    mask = np.asarray(encoder_padding_mask)
    in_maps = []
    for c in range(BS):
        mbc = (mask[c].astype(f) * f(MASK_NEG)).reshape(NT, P).T
        mbc = np.ascontiguousarray(mbc)
        mbsc = np.ascontiguousarray(
            f(SC_C) + f(LOG2E8) * mbc).astype(f)
        xtc = np.ascontiguousarray(hs[c].T) + bo[:, None]
        in_maps.append(dict(
            xt=xtc.astype(bf),
            x8d=np.ascontiguousarray(hs[c].T).astype(f8),
            mb=mbc,
            mbs=mbsc,
            **shared,
        ))
    return in_maps


def _run(in_maps, trace=False):
    nc = _get_nc()
    return run_bass_kernel_spmd(nc, in_maps, list(range(BS)), trace=trace)


def kernel(**inputs):
    in_maps = _prep_in_maps(**inputs)
    res = _run(in_maps, trace=False)
    outs = [res.results[c]["out"].T for c in range(BS)]
    return np.stack(outs, axis=0).astype(np.float32)


def kernel_traced(**inputs):
    in_maps = _prep_in_maps(**inputs)
    res = _run(in_maps, trace=True)
    outs = [res.results[c]["out"].T for c in range(BS)]
    return np.stack(outs, axis=0).astype(np.float32), res.exec_time_ns
